# revision 37
# baseline (speedup 1.0000x reference)
"""Causal attention (B=8, N=4096, D=64) on 8 trn2 NeuronCores.

Sharding: batch b -> core b (data parallel, no cross-core comms).

Per-core kernel (flash-attention style, fully transposed dataflow -- no
on-chip transposes anywhere):
  inputs (host pre-layouts, fp16):
    qk    [64, nqb, 2, 512]  packed (kT | qT) chunks, d on partitions
    v_aug [128, N/128, 65]   k-tiled; col 64 = 1.0; padding-masked rows = 0
    cmasks[128, 4, 512]      causal 0/1 tiles per diagonal alignment
  for each q-block (512 wide), k-tiles grouped in chunks of `chunk` (=3):
    MM1 per tile: logitsT[k, q] = matmul(lhsT=kT_t [64,128], rhs=qT [64,512])
      into one PSUM chunk tile lg [128, chunk, 512].
    exp: ONE ACT op per chunk over [128, len, 512-ls] -> pb (SBUF, fp16),
      where ls (live-column start) skips columns that are entirely above
      the causal diagonal for every tile in the chunk (they get masked to
      zero anyway).  The first pb_bufs chunks run full-width so every pb
      buffer byte is written before it is ever read (NaN guard).
    diagonal tiles: pb[:, i, :] *= causal 0/1 mask (DVE, full width --
      this also zeroes the [0, ls) columns exp skipped).
    MM2 per tile: acc[d+1, q] (+)= matmul(lhsT=v_aug [128,65], rhs=pb)
      -- v_aug col 64 is 1.0 => acc row 64 = the softmax denominators.
  MM2s are emitted one chunk behind the MM1/exp stream so the PE stream is
  [.. MM1s(p) MM2s(p-1) ..] and the exp latency is always hidden.
  Per q-block epilogue: DVE copy acc [65,512] PSUM->SBUF, DMA to
  oT_dram[:, q-block].  NO on-device normalization: the host divides
  rows 0:64 by row 64 (the denominators) and transposes at gather time.
  This removes the old reciprocal/broadcast epilogue chain that stalled
  the ACT queue ~2us per q-block.

PSUM budget: lg [128,3,512] f32 = 3 banks x lg_bufs=2, acc [65,512] f32
= 1 bank x acc_bufs=2  ==> 8 banks exactly.

Input DMAs are issued from two queues in parallel (sync: qk slices in
use order, gpsimd: causal masks + v_aug halves) so the first matmul can
start ~3us earlier than with a single serial issue queue.

Padding mask: host zeroes masked k rows of v_aug (incl. the ones column),
so masked keys contribute nothing to numerator or denominator -- exactly
equivalent to -inf logits.

Matmul operands are fp16 (full rate on the PE; fp32 PSUM accumulation);
measured rel err vs the fp32 reference is ~4e-4.
"""

import os
from contextlib import ExitStack

import numpy as np

B, N, D = 8, 4096, 64
QBLK = 512
KTILE = 128

LAST_RESULTS = None
_NC_CACHE = {}


def build(n=N, d=D, qblk=QBLK, ktile=KTILE, chunk=3, lg_bufs=2, acc_bufs=2,
          pb_bufs=6, warm_mms=14, op_dt="float16"):
    import concourse.bass as bass
    import concourse.mybir as mybir
    import concourse.tile as tile
    from concourse import bacc

    f32 = mybir.dt.float32
    opd = getattr(mybir.dt, op_dt)   # matmul operand dtype
    qblk = min(qblk, n)
    nt = n // ktile          # number of k-tiles
    nqb = n // qblk          # number of q-blocks
    tpq = qblk // ktile      # k-tiles per q-block (diagonal span)

    nc = bacc.Bacc("TRN2", target_bir_lowering=False, debug=False,
                   enable_asserts=False)

    qk_d = nc.dram_tensor("qk", (d, nqb, 2, qblk), opd,
                          kind="ExternalInput").ap()
    v_d = nc.dram_tensor("v_aug", (128, nt, d + 1), opd,
                         kind="ExternalInput").ap()
    mk_d = nc.dram_tensor("cmasks", (128, tpq, qblk), opd,
                          kind="ExternalInput").ap()
    oT_d = nc.dram_tensor("outT", (d + 1, n), f32, kind="ExternalOutput").ap()

    scale = 1.0 / float(np.sqrt(d))

    with tile.TileContext(nc) as tc:
        with ExitStack() as ctx:
            singles = ctx.enter_context(tc.tile_pool(name="singles", bufs=1))
            pb_pool = ctx.enter_context(tc.tile_pool(name="pb", bufs=pb_bufs))
            ob_pool = ctx.enter_context(tc.tile_pool(name="ob", bufs=2))
            lg_pool = ctx.enter_context(
                tc.tile_pool(name="lg", bufs=lg_bufs, space="PSUM"))
            acc_pool = ctx.enter_context(
                tc.tile_pool(name="acc", bufs=acc_bufs, space="PSUM"))

            # --- resident inputs -------------------------------------------
            qk_sb = singles.tile([d, nqb, 2, qblk], opd)
            v_sb = singles.tile([128, nt, d + 1], opd)
            mk_sb = singles.tile([128, tpq, qblk], opd)

            # issue input DMAs from three queues in parallel, critical
            # first.  qb=7 is processed first and consumes kT tiles in
            # ascending order, so it needs its own slice (for qT) plus
            # slice 0 first -- those two go on the scalar queue, which
            # boots ~2us before the sync queue's first DMA slot.
            nc.scalar.dma_start(out=qk_sb[:, nqb - 1, :, :],
                                in_=qk_d[:, nqb - 1, :, :])
            if nqb > 1:
                nc.scalar.dma_start(out=qk_sb[:, 0, :, :],
                                    in_=qk_d[:, 0, :, :])
            for c in range(1, nqb - 1):
                nc.sync.dma_start(out=qk_sb[:, c, :, :], in_=qk_d[:, c, :, :])
            nc.gpsimd.dma_start(out=mk_sb, in_=mk_d)
            half = (nt // 2) if nt > 1 else nt
            nc.gpsimd.dma_start(out=v_sb[:, :half, :], in_=v_d[:, :half, :])
            if half < nt:
                nc.gpsimd.dma_start(out=v_sb[:, half:, :], in_=v_d[:, half:, :])

            def kT_ap(t):
                c, r = divmod(t, tpq)
                return qk_sb[:, c, 0, r * ktile:(r + 1) * ktile]

            # --- PE warm-up ------------------------------------------------
            # The HAM clock gate boots the PE at 1.2 GHz and only releases
            # to 2.4 GHz after ~3.4us of near-continuous matmul activity.
            # Without help, the chunk pipeline's small PE gaps keep the PE
            # cold for the first ~45us (measured).  Emit a burst of
            # dependency-free dummy matmuls (garbage SBUF in, scratch PSUM
            # out, never read) that runs during the input-DMA ramp, so the
            # PE is warm before the first real matmul issues.
            if warm_mms:
                # full 128-partition operands: the HAM activity monitor
                # seems to threshold on array utilization, and the real
                # matmuls (64 contraction rows / 65 out cols) hover at
                # ~50% -- a half-array warm-up burst does not reliably
                # trip it.  18 x 427ns cold also guarantees at least one
                # fully-covered free-running 3.4us HAM window.
                wsrc = singles.tile([128, ktile + qblk], opd)
                nc.vector.memset(wsrc, 0.0)   # idle queue; satisfies the
                wlg = lg_pool.tile([128, chunk, qblk], f32, name="lg",
                                  tag="lg")   # write-before-read rule
                for _ in range(warm_mms):
                    nc.tensor.matmul(
                        wlg[:, 0, :],
                        lhsT=wsrc[:, 0:ktile],
                        rhs=wsrc[:, ktile:],
                        start=True, stop=True,
                    )

            # --- main loop -------------------------------------------------
            # Per chunk: MM1s + one exp + boundary masks, then the deferred
            # MM2s of the previous chunk, so the PE stream interleaves
            # [... MM1s(p) MM2s(p-1) ...] and fills the exp latency.
            mm2_q = []   # deferred MM2s: (acc, pb, t0, length, qb, tlast)
            copy_q = []  # acc->SBUF copies, deferred one further chunk so
                         # the copy never heads the DVE FIFO while waiting
                         # on MM2s (which would block the causal masks
                         # queued behind it and stall the ACT stream).

            def flush_copy():
                acc_, qb_ = copy_q.pop(0)
                ob = ob_pool.tile([d + 1, qblk], f32, name="ob")
                qs = qb_ * qblk
                if qb_ == 0:
                    # final q-block is the serial tail: copy + DMA in halves
                    # so the first DMA overlaps the second copy.
                    h = qblk // 2
                    nc.vector.tensor_copy(ob[:, :h], acc_[:, :h])
                    nc.sync.dma_start(out=oT_d[:, qs:qs + h], in_=ob[:, :h])
                    nc.vector.tensor_copy(ob[:, h:], acc_[:, h:])
                    nc.sync.dma_start(out=oT_d[:, qs + h:qs + qblk],
                                      in_=ob[:, h:])
                else:
                    nc.vector.tensor_copy(ob, acc_)
                    nc.sync.dma_start(out=oT_d[:, qs:qs + qblk], in_=ob)

            def flush_mm2():
                acc_, pb_, t0_, len_, qb_, tlast_ = mm2_q.pop(0)
                for i in range(len_):
                    t = t0_ + i
                    # full width: the extra below-threshold-utilization PE
                    # work doubles as HAM keep-warm activity (restricting
                    # these columns measurably downclocks the PE).
                    nc.tensor.matmul(
                        acc_,
                        lhsT=v_sb[:, t, :],
                        rhs=pb_[:, i, :],
                        start=(t == 0), stop=(t == tlast_),
                    )
                if t0_ + len_ - 1 == tlast_:   # end of q-block: ship raw acc
                    copy_q.append((acc_, qb_))

            # Densest q-block first: qb=7's 10 full chunks saturate the PE
            # with real back-to-back matmuls right from the start, which
            # trips the HAM clock gate to 2.4 GHz within ~2 windows without
            # any dummy warm-up burst.  The thin, diagonal-heavy q-blocks
            # run last, when the warm PE has slack to absorb the causal-
            # mask latency chain.
            nchunks = 0
            for qb in reversed(range(nqb)):
                q_sl = qk_sb[:, qb, 1, :]
                acc = acc_pool.tile([d + 1, qblk], f32, name="acc", tag="acc")
                ntiles = tpq * (qb + 1)
                tlast = ntiles - 1
                # final (smallest) q-block: single-tile chunks so the tail
                # exp->mask->MM2->copy chain pipelines at fine grain
                # instead of serializing behind one fused 3-wide mask.
                csz = 1 if qb == 0 else chunk
                for t0 in range(0, ntiles, csz):
                    while copy_q:   # enqueued in an earlier chunk: its MM2s
                        flush_copy()   # are done, so the DVE never blocks
                    length = min(csz, ntiles - t0)
                    lg = lg_pool.tile([128, chunk, qblk], f32, name="lg",
                                      tag="lg")
                    pb = pb_pool.tile([128, chunk, qblk], opd, name="pb")
                    for i in range(length):
                        nc.tensor.matmul(
                            lg[:, i, :],
                            lhsT=kT_ap(t0 + i),
                            rhs=q_sl,
                            start=True, stop=True,
                        )
                    # live-column start: columns < ls are above the causal
                    # diagonal for every tile in this chunk (the masks
                    # zero them; stale bytes are guarded by the first-use
                    # full-width rounds).
                    ls = max(0, ktile * t0 - qblk * qb)
                    if nchunks < pb_bufs:
                        ls = 0   # first use of each pb buffer: write it all
                    nchunks += 1
                    nc.scalar.activation(
                        pb[:, 0:length, ls:], lg[:, 0:length, ls:],
                        mybir.ActivationFunctionType.Exp,
                        scale=scale)
                    # diagonal tiles of this chunk are consecutive (both in
                    # pb slots and mask alignments): apply all their causal
                    # masks in ONE DVE op to cut op overhead + queue depth.
                    dj = [(i, t0 + i - tpq * qb) for i in range(length)
                          if t0 + i - tpq * qb >= 0]
                    if dj:
                        i0, j0 = dj[0]
                        nmask = len(dj)
                        nc.vector.tensor_mul(
                            pb[:, i0:i0 + nmask, :],
                            pb[:, i0:i0 + nmask, :],
                            mk_sb[:, j0:j0 + nmask, :])
                    mm2_q.append((acc, pb, t0, length, qb, tlast))
                    if len(mm2_q) >= 2:
                        flush_mm2()
            while mm2_q:
                flush_mm2()
            while copy_q:
                flush_copy()

    nc.compile()
    return nc


def _get_nc(key="main", **kw):
    if key not in _NC_CACHE:
        _NC_CACHE[key] = build(**kw)
    return _NC_CACHE[key]


def _prep_core_inputs(q, k, v, attn_mask, b, n=N, d=D, ktile=KTILE,
                      qblk=QBLK, op_dt="float16"):
    npdt = np.float16 if op_dt == "float16" else np.float32
    qblk = min(qblk, n)
    nt = n // ktile
    nqb = n // qblk
    qT = q[b].T.astype(npdt)          # [d, n]
    kT = k[b].T.astype(npdt)
    qk = np.empty((d, nqb, 2, qblk), dtype=npdt)
    qk[:, :, 0, :] = kT.reshape(d, nqb, qblk)
    qk[:, :, 1, :] = qT.reshape(d, nqb, qblk)
    v_aug = np.ones((n, d + 1), dtype=np.float32)
    v_aug[:, :d] = v[b]
    v_aug *= (attn_mask[b] != 0).astype(np.float32)[:, None]
    v_aug = np.ascontiguousarray(
        v_aug.reshape(nt, ktile, d + 1).transpose(1, 0, 2)).astype(npdt)
    tpq = qblk // ktile
    # causal 0/1 mask per diagonal alignment j: keep where q >= k + 128*j
    y = np.arange(qblk)[None, None, :]
    x = np.arange(ktile)[:, None, None]
    jj = np.arange(tpq)[None, :, None]
    cmasks = (y - x - ktile * jj >= 0).astype(npdt)
    return {"qk": qk, "v_aug": v_aug, "cmasks": cmasks}


def kernel(q, k, v, attn_mask):
    global LAST_RESULTS
    q = np.asarray(q, dtype=np.float32)
    k = np.asarray(k, dtype=np.float32)
    v = np.asarray(v, dtype=np.float32)
    attn_mask = np.asarray(attn_mask)

    from concourse.bass_utils import run_bass_kernel_spmd

    nc = _get_nc()
    in_maps = [_prep_core_inputs(q, k, v, attn_mask, b) for b in range(B)]
    trace = bool(os.environ.get("BASS_TRACE"))
    last_err = None
    for attempt in range(3):
        try:
            LAST_RESULTS = run_bass_kernel_spmd(
                nc, in_maps, core_ids=list(range(B)), trace=trace)
            break
        except Exception as e:  # transient device-unrecoverable states clear
            last_err = e        # on the next execution attempt
            if "UNAVAILABLE" not in str(e) and "unrecoverable" not in str(e):
                raise
            import time as _time

            _time.sleep(2.0)
    else:
        raise last_err

    out = np.empty((B, N, D), dtype=np.float32)
    for b in range(B):
        oT = LAST_RESULTS.results[b]["outT"]        # [d+1, n] raw acc
        out[b] = (oT[:D] / oT[D:D + 1]).T           # normalize + transpose
    return out


# revision 38
# speedup vs baseline: 1.0010x; 1.0010x over previous
"""Causal attention (B=8, N=4096, D=64) on 8 trn2 NeuronCores.

Sharding: batch b -> core b (data parallel, no cross-core comms).

Per-core kernel (flash-attention style, fully transposed dataflow -- no
on-chip transposes anywhere):
  inputs (host pre-layouts, fp16):
    qk    [64, nqb, 2, 512]  packed (kT | qT) chunks, d on partitions
    v_aug [128, N/128, 65]   k-tiled; col 64 = 1.0; padding-masked rows = 0
    cmasks[128, 4, 512]      causal 0/1 tiles per diagonal alignment
  for each q-block (512 wide), k-tiles grouped in chunks of `chunk` (=3):
    MM1 per tile: logitsT[k, q] = matmul(lhsT=kT_t [64,128], rhs=qT [64,512])
      into one PSUM chunk tile lg [128, chunk, 512].
    exp: ONE ACT op per chunk over [128, len, 512-ls] -> pb (SBUF, fp16),
      where ls (live-column start) skips columns that are entirely above
      the causal diagonal for every tile in the chunk (they get masked to
      zero anyway).  The first pb_bufs chunks run full-width so every pb
      buffer byte is written before it is ever read (NaN guard).
    diagonal tiles: pb[:, i, :] *= causal 0/1 mask (DVE; all of a chunk's
      masks fused into one op -- this also zeroes the [0, ls) columns exp
      skipped).
    MM2 per tile: acc[d+1, q] (+)= matmul(lhsT=v_aug [128,65], rhs=pb)
      -- v_aug col 64 is 1.0 => acc row 64 = the softmax denominators.
  MM2s are emitted one chunk behind the MM1/exp stream so the PE stream is
  [.. MM1s(p) MM2s(p-1) ..] and the exp latency is always hidden.
  Per q-block epilogue: DVE copy acc [65,512] PSUM->SBUF, DMA to
  oT_dram[:, q-block], deferred to the next chunk's top so the copy never
  heads the DVE FIFO while waiting (that would block the causal masks
  queued behind it and stall the ACT stream).  NO on-device
  normalization: the host divides rows 0:64 by row 64 (the denominators)
  and transposes at gather time -- the old on-device reciprocal/broadcast
  epilogue stalled the ACT queue ~2us per q-block.

The kernel is ACT(exp)-stream-bound (~72.5us of ACTIVATE issue for ~9M
causal exps at 128 lanes x 1.2 GHz); everything else is arranged to keep
that stream gap-free:

* HAM clock gate: the PE boots at 1.2 GHz and its activity monitor only
  releases 2.4 GHz after a ~3.4us window of high-utilization activity.
  The real matmuls (64 contraction rows / 65 output cols) sit at ~50%
  array utilization and do NOT reliably trip it -- measured kernels
  stayed cold for 40+us.  A burst of `warm_mms` dependency-free
  full-array dummy matmuls (128x128 weights, written to a scratch PSUM
  tile nobody reads) runs during the input-DMA ramp and trips the gate
  by ~13us reliably.  Do not "optimize" PE work away (e.g. restricting
  MM2s to live columns): measured, the lower utilization re-cools the
  PE and costs 20-40us.
* q-blocks are processed densest-first (qb=7 .. 0): the dense phase
  right after the warm-up burst keeps the HAM busy-window covered, and
  the thin diagonal-heavy q-blocks run at the end where the warm PE has
  slack to absorb the exp->mask->MM2 latency chain; the final q-block
  uses single-tile chunks and a split copy/DMA so the serial tail is
  short.

PSUM budget: lg [128,3,512] f32 = 3 banks x lg_bufs=2, acc [65,512] f32
= 1 bank x acc_bufs=2  ==> 8 banks exactly.

Input DMAs are issued from three queues in parallel (scalar: the two
first-needed qk slices, sync: remaining qk slices in use order, gpsimd:
causal masks + v_aug halves).

Padding mask: host zeroes masked k rows of v_aug (incl. the ones column),
so masked keys contribute nothing to numerator or denominator -- exactly
equivalent to -inf logits.

Matmul operands are fp16 (full rate on the PE; fp32 PSUM accumulation);
measured rel err vs the fp32 reference is ~4e-4.  Measured ~100-102us
(typical; occasional device-noise outliers) vs the 119.9us baseline.
"""

import os
from contextlib import ExitStack

import numpy as np

B, N, D = 8, 4096, 64
QBLK = 512
KTILE = 128

LAST_RESULTS = None
_NC_CACHE = {}


def build(n=N, d=D, qblk=QBLK, ktile=KTILE, chunk=3, lg_bufs=2, acc_bufs=2,
          pb_bufs=6, warm_mms=14, op_dt="float16"):
    import concourse.bass as bass
    import concourse.mybir as mybir
    import concourse.tile as tile
    from concourse import bacc

    f32 = mybir.dt.float32
    opd = getattr(mybir.dt, op_dt)   # matmul operand dtype
    qblk = min(qblk, n)
    nt = n // ktile          # number of k-tiles
    nqb = n // qblk          # number of q-blocks
    tpq = qblk // ktile      # k-tiles per q-block (diagonal span)

    nc = bacc.Bacc("TRN2", target_bir_lowering=False, debug=False,
                   enable_asserts=False)

    qk_d = nc.dram_tensor("qk", (d, nqb, 2, qblk), opd,
                          kind="ExternalInput").ap()
    v_d = nc.dram_tensor("v_aug", (128, nt, d + 1), opd,
                         kind="ExternalInput").ap()
    mk_d = nc.dram_tensor("cmasks", (128, tpq, qblk), opd,
                          kind="ExternalInput").ap()
    oT_d = nc.dram_tensor("outT", (d + 1, n), f32, kind="ExternalOutput").ap()

    scale = 1.0 / float(np.sqrt(d))

    with tile.TileContext(nc) as tc:
        with ExitStack() as ctx:
            singles = ctx.enter_context(tc.tile_pool(name="singles", bufs=1))
            pb_pool = ctx.enter_context(tc.tile_pool(name="pb", bufs=pb_bufs))
            ob_pool = ctx.enter_context(tc.tile_pool(name="ob", bufs=2))
            lg_pool = ctx.enter_context(
                tc.tile_pool(name="lg", bufs=lg_bufs, space="PSUM"))
            acc_pool = ctx.enter_context(
                tc.tile_pool(name="acc", bufs=acc_bufs, space="PSUM"))

            # --- resident inputs -------------------------------------------
            qk_sb = singles.tile([d, nqb, 2, qblk], opd)
            v_sb = singles.tile([128, nt, d + 1], opd)
            mk_sb = singles.tile([128, tpq, qblk], opd)

            # issue input DMAs from three queues in parallel, critical
            # first.  qb=7 is processed first and consumes kT tiles in
            # ascending order, so it needs its own slice (for qT) plus
            # slice 0 first -- those two go on the scalar queue, which
            # boots ~2us before the sync queue's first DMA slot.
            nc.scalar.dma_start(out=qk_sb[:, nqb - 1, :, :],
                                in_=qk_d[:, nqb - 1, :, :])
            if nqb > 1:
                nc.scalar.dma_start(out=qk_sb[:, 0, :, :],
                                    in_=qk_d[:, 0, :, :])
            for c in range(1, nqb - 1):
                nc.sync.dma_start(out=qk_sb[:, c, :, :], in_=qk_d[:, c, :, :])
            nc.gpsimd.dma_start(out=mk_sb, in_=mk_d)
            half = (nt // 2) if nt > 1 else nt
            nc.gpsimd.dma_start(out=v_sb[:, :half, :], in_=v_d[:, :half, :])
            if half < nt:
                nc.gpsimd.dma_start(out=v_sb[:, half:, :], in_=v_d[:, half:, :])

            def kT_ap(t):
                c, r = divmod(t, tpq)
                return qk_sb[:, c, 0, r * ktile:(r + 1) * ktile]

            # --- PE warm-up ------------------------------------------------
            # The HAM clock gate boots the PE at 1.2 GHz and only releases
            # to 2.4 GHz after ~3.4us of near-continuous matmul activity.
            # Without help, the chunk pipeline's small PE gaps keep the PE
            # cold for the first ~45us (measured).  Emit a burst of
            # dependency-free dummy matmuls (garbage SBUF in, scratch PSUM
            # out, never read) that runs during the input-DMA ramp, so the
            # PE is warm before the first real matmul issues.
            if warm_mms:
                # full 128-partition operands: the HAM activity monitor
                # seems to threshold on array utilization, and the real
                # matmuls (64 contraction rows / 65 out cols) hover at
                # ~50% -- a half-array warm-up burst does not reliably
                # trip it.  18 x 427ns cold also guarantees at least one
                # fully-covered free-running 3.4us HAM window.
                wsrc = singles.tile([128, ktile + qblk], opd)
                nc.vector.memset(wsrc, 0.0)   # idle queue; satisfies the
                wlg = lg_pool.tile([128, chunk, qblk], f32, name="lg",
                                  tag="lg")   # write-before-read rule
                for _ in range(warm_mms):
                    nc.tensor.matmul(
                        wlg[:, 0, :],
                        lhsT=wsrc[:, 0:ktile],
                        rhs=wsrc[:, ktile:],
                        start=True, stop=True,
                    )

            # --- main loop -------------------------------------------------
            # Per chunk: MM1s + one exp + boundary masks, then the deferred
            # MM2s of the previous chunk, so the PE stream interleaves
            # [... MM1s(p) MM2s(p-1) ...] and fills the exp latency.
            mm2_q = []   # deferred MM2s: (acc, pb, t0, length, qb, tlast)
            copy_q = []  # acc->SBUF copies, deferred one further chunk so
                         # the copy never heads the DVE FIFO while waiting
                         # on MM2s (which would block the causal masks
                         # queued behind it and stall the ACT stream).

            def flush_copy():
                acc_, qb_ = copy_q.pop(0)
                ob = ob_pool.tile([d + 1, qblk], f32, name="ob")
                qs = qb_ * qblk
                if qb_ == 0:
                    # final q-block is the serial tail: copy + DMA in halves
                    # so the first DMA overlaps the second copy.
                    h = qblk // 2
                    nc.vector.tensor_copy(ob[:, :h], acc_[:, :h])
                    nc.sync.dma_start(out=oT_d[:, qs:qs + h], in_=ob[:, :h])
                    nc.vector.tensor_copy(ob[:, h:], acc_[:, h:])
                    nc.sync.dma_start(out=oT_d[:, qs + h:qs + qblk],
                                      in_=ob[:, h:])
                else:
                    nc.vector.tensor_copy(ob, acc_)
                    nc.sync.dma_start(out=oT_d[:, qs:qs + qblk], in_=ob)

            def flush_mm2():
                acc_, pb_, t0_, len_, qb_, tlast_ = mm2_q.pop(0)
                for i in range(len_):
                    t = t0_ + i
                    # full width: the extra below-threshold-utilization PE
                    # work doubles as HAM keep-warm activity (restricting
                    # these columns measurably downclocks the PE).
                    nc.tensor.matmul(
                        acc_,
                        lhsT=v_sb[:, t, :],
                        rhs=pb_[:, i, :],
                        start=(t == 0), stop=(t == tlast_),
                    )
                if t0_ + len_ - 1 == tlast_:   # end of q-block: ship raw acc
                    copy_q.append((acc_, qb_))

            # Densest q-block first: qb=7's 10 full chunks saturate the PE
            # with real back-to-back matmuls right from the start, which
            # trips the HAM clock gate to 2.4 GHz within ~2 windows without
            # any dummy warm-up burst.  The thin, diagonal-heavy q-blocks
            # run last, when the warm PE has slack to absorb the causal-
            # mask latency chain.
            nchunks = 0
            for qb in reversed(range(nqb)):
                q_sl = qk_sb[:, qb, 1, :]
                acc = acc_pool.tile([d + 1, qblk], f32, name="acc", tag="acc")
                ntiles = tpq * (qb + 1)
                tlast = ntiles - 1
                # final (smallest) q-block: single-tile chunks so the tail
                # exp->mask->MM2->copy chain pipelines at fine grain
                # instead of serializing behind one fused 3-wide mask.
                csz = 1 if qb == 0 else chunk
                for t0 in range(0, ntiles, csz):
                    while copy_q:   # enqueued in an earlier chunk: its MM2s
                        flush_copy()   # are done, so the DVE never blocks
                    length = min(csz, ntiles - t0)
                    lg = lg_pool.tile([128, chunk, qblk], f32, name="lg",
                                      tag="lg")
                    pb = pb_pool.tile([128, chunk, qblk], opd, name="pb")
                    for i in range(length):
                        nc.tensor.matmul(
                            lg[:, i, :],
                            lhsT=kT_ap(t0 + i),
                            rhs=q_sl,
                            start=True, stop=True,
                        )
                    # live-column start: columns < ls are above the causal
                    # diagonal for every tile in this chunk (the masks
                    # zero them; stale bytes are guarded by the first-use
                    # full-width rounds).
                    ls = max(0, ktile * t0 - qblk * qb)
                    if nchunks < pb_bufs:
                        ls = 0   # first use of each pb buffer: write it all
                    nchunks += 1
                    nc.scalar.activation(
                        pb[:, 0:length, ls:], lg[:, 0:length, ls:],
                        mybir.ActivationFunctionType.Exp,
                        scale=scale)
                    # diagonal tiles of this chunk are consecutive (both in
                    # pb slots and mask alignments): apply all their causal
                    # masks in ONE DVE op to cut op overhead + queue depth.
                    dj = [(i, t0 + i - tpq * qb) for i in range(length)
                          if t0 + i - tpq * qb >= 0]
                    if dj:
                        i0, j0 = dj[0]
                        nmask = len(dj)
                        nc.vector.tensor_mul(
                            pb[:, i0:i0 + nmask, :],
                            pb[:, i0:i0 + nmask, :],
                            mk_sb[:, j0:j0 + nmask, :])
                    mm2_q.append((acc, pb, t0, length, qb, tlast))
                    if len(mm2_q) >= 2:
                        flush_mm2()
            while mm2_q:
                flush_mm2()
            while copy_q:
                flush_copy()

    nc.compile()
    return nc


def _get_nc(key="main", **kw):
    if key not in _NC_CACHE:
        _NC_CACHE[key] = build(**kw)
    return _NC_CACHE[key]


def _prep_core_inputs(q, k, v, attn_mask, b, n=N, d=D, ktile=KTILE,
                      qblk=QBLK, op_dt="float16"):
    npdt = np.float16 if op_dt == "float16" else np.float32
    qblk = min(qblk, n)
    nt = n // ktile
    nqb = n // qblk
    qT = q[b].T.astype(npdt)          # [d, n]
    kT = k[b].T.astype(npdt)
    qk = np.empty((d, nqb, 2, qblk), dtype=npdt)
    qk[:, :, 0, :] = kT.reshape(d, nqb, qblk)
    qk[:, :, 1, :] = qT.reshape(d, nqb, qblk)
    v_aug = np.ones((n, d + 1), dtype=np.float32)
    v_aug[:, :d] = v[b]
    v_aug *= (attn_mask[b] != 0).astype(np.float32)[:, None]
    v_aug = np.ascontiguousarray(
        v_aug.reshape(nt, ktile, d + 1).transpose(1, 0, 2)).astype(npdt)
    tpq = qblk // ktile
    # causal 0/1 mask per diagonal alignment j: keep where q >= k + 128*j
    y = np.arange(qblk)[None, None, :]
    x = np.arange(ktile)[:, None, None]
    jj = np.arange(tpq)[None, :, None]
    cmasks = (y - x - ktile * jj >= 0).astype(npdt)
    return {"qk": qk, "v_aug": v_aug, "cmasks": cmasks}


def kernel(q, k, v, attn_mask):
    global LAST_RESULTS
    q = np.asarray(q, dtype=np.float32)
    k = np.asarray(k, dtype=np.float32)
    v = np.asarray(v, dtype=np.float32)
    attn_mask = np.asarray(attn_mask)

    from concourse.bass_utils import run_bass_kernel_spmd

    nc = _get_nc()
    in_maps = [_prep_core_inputs(q, k, v, attn_mask, b) for b in range(B)]
    trace = bool(os.environ.get("BASS_TRACE"))
    last_err = None
    for attempt in range(3):
        try:
            LAST_RESULTS = run_bass_kernel_spmd(
                nc, in_maps, core_ids=list(range(B)), trace=trace)
            break
        except Exception as e:  # transient device-unrecoverable states clear
            last_err = e        # on the next execution attempt
            if "UNAVAILABLE" not in str(e) and "unrecoverable" not in str(e):
                raise
            import time as _time

            _time.sleep(2.0)
    else:
        raise last_err

    out = np.empty((B, N, D), dtype=np.float32)
    for b in range(B):
        oT = LAST_RESULTS.results[b]["outT"]        # [d+1, n] raw acc
        out[b] = (oT[:D] / oT[D:D + 1]).T           # normalize + transpose
    return out
